# revision 7
# baseline (speedup 1.0000x reference)
import sys

if "/opt/trn_rl_repo" not in sys.path:
    sys.path.insert(0, "/opt/trn_rl_repo")

import numpy as np

from concourse import bacc, mybir, tile
from concourse.bass_utils import run_bass_kernel_spmd

N_CORES = 8
B, C, H, W = 4096, 2, 64, 64
BPC = B // N_CORES          # 512 batches per core
NS = BPC // 16              # 32 supertiles of 16 maps each
NCHUNK = 16                 # data-loss chunks of [128, 2048] per tensor
CHUNK_F = 2048
GRID_D = 1.0 / (H - 1)
CLAMP_NEG_MIN = 27.6310211159  # -CLAMP_MIN

F32 = mybir.dt.float32


def _d1_matrix(n, d):
    m = np.zeros((n, n), dtype=np.float64)
    for i in range(1, n - 1):
        m[i, i - 1], m[i, i + 1] = -1.0, 1.0
    m[0, 0], m[0, 1], m[0, 2] = -3.0, 4.0, -1.0
    m[-1, -1], m[-1, -2], m[-1, -3] = 3.0, -4.0, 1.0
    return m / (2.0 * d)


def _d2_matrix(n, d):
    m = np.zeros((n, n), dtype=np.float64)
    for i in range(1, n - 1):
        m[i, i - 1], m[i, i], m[i, i + 1] = 1.0, -2.0, 1.0
    m[0, 0:4] = [2.0, -5.0, 4.0, -1.0]
    m[-1, -1], m[-1, -2], m[-1, -3], m[-1, -4] = 2.0, -5.0, 4.0, -1.0
    return m / (d * d)


def _build_consts():
    d1 = _d1_matrix(H, GRID_D)
    d2 = _d2_matrix(H, GRID_D)
    e = -(d2 + d1.T @ d1)            # sum(perm*(E@p)) == -sum(perm*d2p) - sum(d1perm*d1p)
    g = d1[H - 1, :] - d1[0, :]      # Neumann-boundary row functional

    # lhsT for the E matmul: out = lhsT.T @ rhs must be blkdiag(E, E) @ rhs
    c_e = np.zeros((128, 128), dtype=np.float32)
    c_e[0:64, 0:64] = e.T.astype(np.float32)
    c_e[64:128, 64:128] = e.T.astype(np.float32)

    c_i = np.eye(128, dtype=np.float32)

    # Banded reduction weights: slicing cols [63-2s : 127-2s] of this gives a
    # [128, 64] lhsT whose only nonzero columns are 2s (partitions 0:64) and
    # 2s+1 (partitions 64:128) — so supertile s's partition-sums land in PSUM
    # rows 2s, 2s+1 while start=False accumulation leaves other rows untouched.
    # bf16: ones are exact, and bf16 matmuls stream 4x faster than fp32.
    import ml_dtypes

    c_ones = np.zeros((128, 128), dtype=ml_dtypes.bfloat16)
    for p in range(128):
        c_ones[p, 63 + p // 64] = 1.0

    # Boundary fold: sum((perm + a 1^T) (.) E p) = sum(perm (.) E p) + g^T rowsums(p)
    # when E^T a = g, so the Neumann boundary terms ride the same product/reduce.
    a = np.linalg.lstsq(e.T, g, rcond=None)[0]
    assert np.abs(e.T @ a - g).max() < 1e-9
    c_a = np.zeros((128, 2), dtype=np.float32)
    c_a[:, 0] = np.tile(a, 2).astype(np.float32)
    c_a[:, 1] = -c_a[:, 0]

    return {"cE": c_e, "cI": c_i, "cOnes": c_ones, "cA": c_a}


def _build_nc():
    nc = bacc.Bacc("TRN2", target_bir_lowering=False, debug=False)

    x0 = nc.dram_tensor("x0", [NS, 16, 2, H, W], F32, kind="ExternalInput")
    mo = nc.dram_tensor("mo", [NCHUNK, 128, CHUNK_F], F32, kind="ExternalInput")
    tg = nc.dram_tensor("tg", [NCHUNK, 128, CHUNK_F], F32, kind="ExternalInput")
    c_e = nc.dram_tensor("cE", [128, 128], F32, kind="ExternalInput")
    c_i = nc.dram_tensor("cI", [128, 128], F32, kind="ExternalInput")
    c_ones = nc.dram_tensor(
        "cOnes", [128, 128], mybir.dt.bfloat16, kind="ExternalInput"
    )
    c_a = nc.dram_tensor("cA", [128, 2], F32, kind="ExternalInput")

    s1_out = nc.dram_tensor("s1", [64, 8], F32, kind="ExternalOutput")
    s2_out = nc.dram_tensor("s2", [64, 8], F32, kind="ExternalOutput")
    dstat_out = nc.dram_tensor("dstat", [128, NCHUNK], F32, kind="ExternalOutput")

    with tile.TileContext(nc) as tc:
        with (
            tc.tile_pool(name="consts", bufs=1) as cpool,
            tc.tile_pool(name="inp", bufs=4) as ipool,
            tc.tile_pool(name="work", bufs=2) as wpool,
            tc.tile_pool(name="dchunk", bufs=4) as dpool,
            tc.tile_pool(name="stats", bufs=1) as stpool,
            tc.tile_pool(name="pwork", bufs=2, space="PSUM") as pwpool,
            tc.tile_pool(name="ptrans", bufs=1, space="PSUM") as ptpool,
            tc.tile_pool(name="paccum", bufs=1, space="PSUM") as papool,
        ):
            ce = cpool.tile([128, 128], F32, tag="ce")
            ci = cpool.tile([128, 128], F32, tag="ci")
            cones = cpool.tile([128, 128], mybir.dt.bfloat16, tag="cones")
            ca = cpool.tile([128, 2], F32, tag="ca")
            nc.sync.dma_start(ce[:], c_e[:])
            nc.sync.dma_start(ci[:], c_i[:])
            nc.sync.dma_start(cones[:], c_ones[:])
            nc.sync.dma_start(ca[:], c_a[:])

            sall = papool.tile([64, 512], F32, tag="sall")
            st = papool.tile([64, 512], F32, tag="st")
            dstat = stpool.tile([128, NCHUNK], F32, tag="dstat")

            for s in range(NS):
                p_t = ipool.tile([128, 512], F32, tag="p")
                perm_t = ipool.tile([128, 512], F32, tag="perm")
                # supertile layout: partition 64*r + h, free 64*j + w holds
                # batch 16*s + 8*r + j (channel 0 -> p_t, channel 1 -> perm_t)
                for r in range(2):
                    for j in range(8):
                        b = 8 * r + j
                        nc.sync.dma_start(
                            p_t[64 * r : 64 * r + 64, 64 * j : 64 * j + 64],
                            x0[s, b, 0],
                        )
                        nc.sync.dma_start(
                            perm_t[64 * r : 64 * r + 64, 64 * j : 64 * j + 64],
                            x0[s, b, 1],
                        )

                ptp = ptpool.tile([128, 512], F32, tag="ptp")
                permtp = ptpool.tile([128, 512], F32, tag="permtp")
                for k in range(4):
                    nc.tensor.transpose(
                        ptp[:, 128 * k : 128 * (k + 1)],
                        p_t[:, 128 * k : 128 * (k + 1)],
                        ci[:],
                    )
                    nc.tensor.transpose(
                        permtp[:, 128 * k : 128 * (k + 1)],
                        perm_t[:, 128 * k : 128 * (k + 1)],
                        ci[:],
                    )
                pt_s = wpool.tile([128, 512], F32, tag="pt")
                permt_s = wpool.tile([128, 512], F32, tag="permt")
                nc.scalar.copy(pt_s[:], ptp[:])
                nc.scalar.copy(permt_s[:], permtp[:])

                ep = pwpool.tile([128, 512], F32, tag="ep")
                ept = pwpool.tile([128, 512], F32, tag="ept")
                nc.tensor.matmul(ep[:], ce[:], p_t[:], start=True, stop=True)
                nc.tensor.matmul(ept[:], ce[:], pt_s[:], start=True, stop=True)

                u1 = wpool.tile([128, 512], mybir.dt.bfloat16, tag="u1")
                u2 = wpool.tile([128, 512], mybir.dt.bfloat16, tag="u2")
                nc.vector.scalar_tensor_tensor(
                    u1[:], perm_t[:], ca[:, 0:1], ep[:],
                    op0=mybir.AluOpType.add, op1=mybir.AluOpType.mult,
                )
                nc.vector.scalar_tensor_tensor(
                    u2[:], permt_s[:], ca[:, 1:2], ept[:],
                    op0=mybir.AluOpType.add, op1=mybir.AluOpType.mult,
                )

                # per-(map,col) partition sums accumulated into persistent PSUM
                # rows 2s, 2s+1 via the banded lhsT slice
                lo, hi = 63 - 2 * s, 127 - 2 * s
                first, last = s == 0, s == NS - 1
                nc.tensor.matmul(
                    sall[:], cones[:, lo:hi], u1[:],
                    start=first, stop=last, skip_group_check=True,
                )
                nc.tensor.matmul(
                    st[:], cones[:, lo:hi], u2[:],
                    start=first, stop=last, skip_group_check=True,
                )

                # data loss: one [128, 2048] chunk every other supertile;
                # subtract alternates DVE/GpSimd to balance engine load
                if s % 2 == 0:
                    k = s // 2
                    mt = dpool.tile([128, CHUNK_F], F32, tag="mt")
                    tt = dpool.tile([128, CHUNK_F], F32, tag="tt")
                    nc.sync.dma_start(mt[:], mo[k])
                    nc.sync.dma_start(tt[:], tg[k])
                    eng = nc.vector if k % 2 == 0 else nc.gpsimd
                    eng.tensor_sub(mt[:], mt[:], tt[:])
                    nc.scalar.activation(
                        mt[:],
                        mt[:],
                        mybir.ActivationFunctionType.Square,
                        accum_out=dstat[:, k : k + 1],
                    )

            s1_t = stpool.tile([64, 8], F32, tag="s1t")
            s2_t = stpool.tile([64, 8], F32, tag="s2t")
            nc.vector.reduce_sum(
                s1_t[:],
                sall[:].rearrange("p (j w) -> p j w", j=8),
                axis=mybir.AxisListType.X,
            )
            nc.vector.reduce_sum(
                s2_t[:],
                st[:].rearrange("p (j w) -> p j w", j=8),
                axis=mybir.AxisListType.X,
            )
            nc.sync.dma_start(s1_out[:], s1_t[:])
            nc.sync.dma_start(s2_out[:], s2_t[:])
            nc.sync.dma_start(dstat_out[:], dstat[:])

    nc.compile()
    return nc


_NC = None
_CONSTS = None
LAST_RESULTS = None


def kernel(model_out, target, x0_hat, var, _trace=False, _trace_kwargs=None):
    global _NC, _CONSTS, LAST_RESULTS
    if _NC is None:
        _CONSTS = _build_consts()
        _NC = _build_nc()

    model_out = np.ascontiguousarray(model_out, dtype=np.float32)
    target = np.ascontiguousarray(target, dtype=np.float32)
    x0_hat = np.ascontiguousarray(x0_hat, dtype=np.float32)
    var = np.asarray(var, dtype=np.float32)

    in_maps = []
    for c in range(N_CORES):
        lo, hi = c * BPC, (c + 1) * BPC
        in_maps.append(
            {
                "x0": x0_hat[lo:hi].reshape(NS, 16, 2, H, W),
                "mo": model_out[lo:hi].reshape(NCHUNK, 128, CHUNK_F),
                "tg": target[lo:hi].reshape(NCHUNK, 128, CHUNK_F),
                **_CONSTS,
            }
        )

    kwargs = {}
    if _trace:
        kwargs["trace"] = True
        if _trace_kwargs:
            kwargs.update(_trace_kwargs)
    res = run_bass_kernel_spmd(_NC, in_maps, list(range(N_CORES)), **kwargs)
    LAST_RESULTS = res

    data_sum = 0.0
    nll_sum = 0.0
    for c in range(N_CORES):
        out = res.results[c]
        s1 = out["s1"].astype(np.float64)       # [64, 8]
        s2 = out["s2"].astype(np.float64)       # [64, 8]
        dstat = out["dstat"].astype(np.float64)  # [128, 16]

        # s1[2s+r, j] -> batch 16s + 8r + j
        r1 = s1.reshape(NS, 2, 8).reshape(BPC)
        # s2[2s+x, 2k+y] -> batch 16s + 8y + 2k + x
        r2 = s2.reshape(NS, 2, 4, 2).transpose(0, 3, 2, 1).reshape(BPC)
        r = (r1 + r2) / (H * W * 3.0)

        v = var[c * BPC : (c + 1) * BPC].astype(np.float64)
        nll = np.minimum(0.5 * r * r / v, CLAMP_NEG_MIN)
        nll_sum += nll.sum()
        data_sum += dstat.sum()

    loss = data_sum / (B * C * H * W) + nll_sum / B
    return np.float32(loss)


# revision 8
# speedup vs baseline: 3.1077x; 3.1077x over previous
import sys

if "/opt/trn_rl_repo" not in sys.path:
    sys.path.insert(0, "/opt/trn_rl_repo")

import numpy as np

from concourse import bacc, mybir, tile
from concourse.bass_utils import run_bass_kernel_spmd

N_CORES = 8
B, C, H, W = 4096, 2, 64, 64
BPC = B // N_CORES          # 512 batches per core
NS = BPC // 16              # 32 supertiles of 16 maps each
NCHUNK = 16                 # data-loss chunks of [128, 2048] per tensor
CHUNK_F = 2048
GRID_D = 1.0 / (H - 1)
CLAMP_NEG_MIN = 27.6310211159  # -CLAMP_MIN

F32 = mybir.dt.float32


def _d1_matrix(n, d):
    m = np.zeros((n, n), dtype=np.float64)
    for i in range(1, n - 1):
        m[i, i - 1], m[i, i + 1] = -1.0, 1.0
    m[0, 0], m[0, 1], m[0, 2] = -3.0, 4.0, -1.0
    m[-1, -1], m[-1, -2], m[-1, -3] = 3.0, -4.0, 1.0
    return m / (2.0 * d)


def _d2_matrix(n, d):
    m = np.zeros((n, n), dtype=np.float64)
    for i in range(1, n - 1):
        m[i, i - 1], m[i, i], m[i, i + 1] = 1.0, -2.0, 1.0
    m[0, 0:4] = [2.0, -5.0, 4.0, -1.0]
    m[-1, -1], m[-1, -2], m[-1, -3], m[-1, -4] = 2.0, -5.0, 4.0, -1.0
    return m / (d * d)


def _build_consts():
    d1 = _d1_matrix(H, GRID_D)
    d2 = _d2_matrix(H, GRID_D)
    e = -(d2 + d1.T @ d1)            # sum(perm*(E@p)) == -sum(perm*d2p) - sum(d1perm*d1p)
    g = d1[H - 1, :] - d1[0, :]      # Neumann-boundary row functional

    # lhsT for the E matmul: out = lhsT.T @ rhs must be blkdiag(E, E) @ rhs
    c_e = np.zeros((128, 128), dtype=np.float32)
    c_e[0:64, 0:64] = e.T.astype(np.float32)
    c_e[64:128, 64:128] = e.T.astype(np.float32)

    c_i = np.eye(128, dtype=np.float32)

    # Banded reduction weights: slicing cols [63-2s : 127-2s] of this gives a
    # [128, 64] lhsT whose only nonzero columns are 2s (partitions 0:64) and
    # 2s+1 (partitions 64:128) — so supertile s's partition-sums land in PSUM
    # rows 2s, 2s+1 while start=False accumulation leaves other rows untouched.
    # bf16: ones are exact, and bf16 matmuls stream 4x faster than fp32.
    import ml_dtypes

    c_ones = np.zeros((128, 128), dtype=ml_dtypes.bfloat16)
    for p in range(128):
        c_ones[p, 63 + p // 64] = 1.0

    # Boundary fold: sum((perm + a 1^T) (.) E p) = sum(perm (.) E p) + g^T rowsums(p)
    # when E^T a = g, so the Neumann boundary terms ride the same product/reduce.
    a = np.linalg.lstsq(e.T, g, rcond=None)[0]
    assert np.abs(e.T @ a - g).max() < 1e-9
    c_a = np.zeros((128, 2), dtype=np.float32)
    c_a[:, 0] = np.tile(a, 2).astype(np.float32)
    c_a[:, 1] = -c_a[:, 0]

    return {"cE": c_e, "cI": c_i, "cOnes": c_ones, "cA": c_a}


def _build_nc():
    nc = bacc.Bacc("TRN2", target_bir_lowering=False, debug=False)

    x0 = nc.dram_tensor("x0", [NS, 16, 2, H, W], F32, kind="ExternalInput")
    mo = nc.dram_tensor("mo", [NCHUNK, 128, CHUNK_F], F32, kind="ExternalInput")
    tg = nc.dram_tensor("tg", [NCHUNK, 128, CHUNK_F], F32, kind="ExternalInput")
    c_e = nc.dram_tensor("cE", [128, 128], F32, kind="ExternalInput")
    c_i = nc.dram_tensor("cI", [128, 128], F32, kind="ExternalInput")
    c_ones = nc.dram_tensor(
        "cOnes", [128, 128], mybir.dt.bfloat16, kind="ExternalInput"
    )
    c_a = nc.dram_tensor("cA", [128, 2], F32, kind="ExternalInput")

    s1_out = nc.dram_tensor("s1", [64, 8], F32, kind="ExternalOutput")
    s2_out = nc.dram_tensor("s2", [64, 8], F32, kind="ExternalOutput")
    dstat_out = nc.dram_tensor("dstat", [128, NCHUNK], F32, kind="ExternalOutput")

    with tile.TileContext(nc) as tc:
        with (
            tc.tile_pool(name="consts", bufs=1) as cpool,
            tc.tile_pool(name="inp", bufs=4) as ipool,
            tc.tile_pool(name="work", bufs=2) as wpool,
            tc.tile_pool(name="dchunk", bufs=4) as dpool,
            tc.tile_pool(name="stats", bufs=1) as stpool,
            tc.tile_pool(name="pwork", bufs=2, space="PSUM") as pwpool,
            tc.tile_pool(name="ptrans", bufs=1, space="PSUM") as ptpool,
            tc.tile_pool(name="paccum", bufs=1, space="PSUM") as papool,
        ):
            ce = cpool.tile([128, 128], F32, tag="ce")
            ci = cpool.tile([128, 128], F32, tag="ci")
            cones = cpool.tile([128, 128], mybir.dt.bfloat16, tag="cones")
            ca = cpool.tile([128, 2], F32, tag="ca")
            nc.sync.dma_start(ce[:], c_e[:])
            nc.sync.dma_start(ci[:], c_i[:])
            nc.sync.dma_start(cones[:], c_ones[:])
            nc.sync.dma_start(ca[:], c_a[:])

            sall = papool.tile([64, 512], F32, tag="sall")
            st = papool.tile([64, 512], F32, tag="st")
            dstat = stpool.tile([128, NCHUNK], F32, tag="dstat")

            for s in range(NS):
                p_t = ipool.tile([128, 512], F32, tag="p")
                perm_t = ipool.tile([128, 512], F32, tag="perm")
                # supertile layout: partition 64*r + h, free 64*j + w holds
                # batch 16*s + 8*r + j (channel 0 -> p_t, channel 1 -> perm_t)
                for r in range(2):
                    nc.sync.dma_start(
                        p_t[64 * r : 64 * (r + 1), :].rearrange(
                            "h (j w) -> h j w", j=8
                        ),
                        x0[s, 8 * r : 8 * r + 8, 0].rearrange("j h w -> h j w"),
                    )
                    nc.sync.dma_start(
                        perm_t[64 * r : 64 * (r + 1), :].rearrange(
                            "h (j w) -> h j w", j=8
                        ),
                        x0[s, 8 * r : 8 * r + 8, 1].rearrange("j h w -> h j w"),
                    )

                ptp = ptpool.tile([128, 512], F32, tag="ptp")
                permtp = ptpool.tile([128, 512], F32, tag="permtp")
                for k in range(4):
                    nc.tensor.transpose(
                        ptp[:, 128 * k : 128 * (k + 1)],
                        p_t[:, 128 * k : 128 * (k + 1)],
                        ci[:],
                    )
                    nc.tensor.transpose(
                        permtp[:, 128 * k : 128 * (k + 1)],
                        perm_t[:, 128 * k : 128 * (k + 1)],
                        ci[:],
                    )
                pt_s = wpool.tile([128, 512], F32, tag="pt")
                permt_s = wpool.tile([128, 512], F32, tag="permt")
                nc.scalar.copy(pt_s[:], ptp[:])
                nc.scalar.copy(permt_s[:], permtp[:])

                ep = pwpool.tile([128, 512], F32, tag="ep")
                ept = pwpool.tile([128, 512], F32, tag="ept")
                nc.tensor.matmul(ep[:], ce[:], p_t[:], start=True, stop=True)
                nc.tensor.matmul(ept[:], ce[:], pt_s[:], start=True, stop=True)

                u1 = wpool.tile([128, 512], mybir.dt.bfloat16, tag="u1")
                u2 = wpool.tile([128, 512], mybir.dt.bfloat16, tag="u2")
                nc.vector.scalar_tensor_tensor(
                    u1[:], perm_t[:], ca[:, 0:1], ep[:],
                    op0=mybir.AluOpType.add, op1=mybir.AluOpType.mult,
                )
                nc.vector.scalar_tensor_tensor(
                    u2[:], permt_s[:], ca[:, 1:2], ept[:],
                    op0=mybir.AluOpType.add, op1=mybir.AluOpType.mult,
                )

                # per-(map,col) partition sums accumulated into persistent PSUM
                # rows 2s, 2s+1 via the banded lhsT slice
                lo, hi = 63 - 2 * s, 127 - 2 * s
                first, last = s == 0, s == NS - 1
                nc.tensor.matmul(
                    sall[:], cones[:, lo:hi], u1[:],
                    start=first, stop=last, skip_group_check=True,
                )
                nc.tensor.matmul(
                    st[:], cones[:, lo:hi], u2[:],
                    start=first, stop=last, skip_group_check=True,
                )

                # data loss: one [128, 2048] chunk every other supertile;
                # subtract alternates DVE/GpSimd to balance engine load
                if s % 2 == 0:
                    k = s // 2
                    mt = dpool.tile([128, CHUNK_F], F32, tag="mt")
                    tt = dpool.tile([128, CHUNK_F], F32, tag="tt")
                    nc.sync.dma_start(mt[:], mo[k])
                    nc.sync.dma_start(tt[:], tg[k])
                    eng = nc.vector if k % 2 == 0 else nc.gpsimd
                    eng.tensor_sub(mt[:], mt[:], tt[:])
                    nc.scalar.activation(
                        mt[:],
                        mt[:],
                        mybir.ActivationFunctionType.Square,
                        accum_out=dstat[:, k : k + 1],
                    )

            s1_t = stpool.tile([64, 8], F32, tag="s1t")
            s2_t = stpool.tile([64, 8], F32, tag="s2t")
            nc.vector.reduce_sum(
                s1_t[:],
                sall[:].rearrange("p (j w) -> p j w", j=8),
                axis=mybir.AxisListType.X,
            )
            nc.vector.reduce_sum(
                s2_t[:],
                st[:].rearrange("p (j w) -> p j w", j=8),
                axis=mybir.AxisListType.X,
            )
            nc.sync.dma_start(s1_out[:], s1_t[:])
            nc.sync.dma_start(s2_out[:], s2_t[:])
            nc.sync.dma_start(dstat_out[:], dstat[:])

    nc.compile()
    return nc


_NC = None
_CONSTS = None
LAST_RESULTS = None


def kernel(model_out, target, x0_hat, var, _trace=False, _trace_kwargs=None):
    global _NC, _CONSTS, LAST_RESULTS
    if _NC is None:
        _CONSTS = _build_consts()
        _NC = _build_nc()

    model_out = np.ascontiguousarray(model_out, dtype=np.float32)
    target = np.ascontiguousarray(target, dtype=np.float32)
    x0_hat = np.ascontiguousarray(x0_hat, dtype=np.float32)
    var = np.asarray(var, dtype=np.float32)

    in_maps = []
    for c in range(N_CORES):
        lo, hi = c * BPC, (c + 1) * BPC
        in_maps.append(
            {
                "x0": x0_hat[lo:hi].reshape(NS, 16, 2, H, W),
                "mo": model_out[lo:hi].reshape(NCHUNK, 128, CHUNK_F),
                "tg": target[lo:hi].reshape(NCHUNK, 128, CHUNK_F),
                **_CONSTS,
            }
        )

    kwargs = {}
    if _trace:
        kwargs["trace"] = True
        if _trace_kwargs:
            kwargs.update(_trace_kwargs)
    res = run_bass_kernel_spmd(_NC, in_maps, list(range(N_CORES)), **kwargs)
    LAST_RESULTS = res

    data_sum = 0.0
    nll_sum = 0.0
    for c in range(N_CORES):
        out = res.results[c]
        s1 = out["s1"].astype(np.float64)       # [64, 8]
        s2 = out["s2"].astype(np.float64)       # [64, 8]
        dstat = out["dstat"].astype(np.float64)  # [128, 16]

        # s1[2s+r, j] -> batch 16s + 8r + j
        r1 = s1.reshape(NS, 2, 8).reshape(BPC)
        # s2[2s+x, 2k+y] -> batch 16s + 8y + 2k + x
        r2 = s2.reshape(NS, 2, 4, 2).transpose(0, 3, 2, 1).reshape(BPC)
        r = (r1 + r2) / (H * W * 3.0)

        v = var[c * BPC : (c + 1) * BPC].astype(np.float64)
        nll = np.minimum(0.5 * r * r / v, CLAMP_NEG_MIN)
        nll_sum += nll.sum()
        data_sum += dstat.sum()

    loss = data_sum / (B * C * H * W) + nll_sum / B
    return np.float32(loss)


# revision 9
# speedup vs baseline: 4.2782x; 1.3766x over previous
import sys

if "/opt/trn_rl_repo" not in sys.path:
    sys.path.insert(0, "/opt/trn_rl_repo")

import numpy as np

from concourse import bacc, mybir, tile
from concourse.bass_utils import run_bass_kernel_spmd

N_CORES = 8
B, C, H, W = 4096, 2, 64, 64
BPC = B // N_CORES          # 512 batches per core
NS = BPC // 16              # 32 supertiles of 16 maps each
NCHUNK = 16                 # data-loss chunks of [128, 2048] per tensor
CHUNK_F = 2048
GRID_D = 1.0 / (H - 1)
CLAMP_NEG_MIN = 27.6310211159  # -CLAMP_MIN

F32 = mybir.dt.float32


def _d1_matrix(n, d):
    m = np.zeros((n, n), dtype=np.float64)
    for i in range(1, n - 1):
        m[i, i - 1], m[i, i + 1] = -1.0, 1.0
    m[0, 0], m[0, 1], m[0, 2] = -3.0, 4.0, -1.0
    m[-1, -1], m[-1, -2], m[-1, -3] = 3.0, -4.0, 1.0
    return m / (2.0 * d)


def _d2_matrix(n, d):
    m = np.zeros((n, n), dtype=np.float64)
    for i in range(1, n - 1):
        m[i, i - 1], m[i, i], m[i, i + 1] = 1.0, -2.0, 1.0
    m[0, 0:4] = [2.0, -5.0, 4.0, -1.0]
    m[-1, -1], m[-1, -2], m[-1, -3], m[-1, -4] = 2.0, -5.0, 4.0, -1.0
    return m / (d * d)


def _build_consts():
    d1 = _d1_matrix(H, GRID_D)
    d2 = _d2_matrix(H, GRID_D)
    e = -(d2 + d1.T @ d1)            # sum(perm*(E@p)) == -sum(perm*d2p) - sum(d1perm*d1p)
    g = d1[H - 1, :] - d1[0, :]      # Neumann-boundary row functional

    # lhsT for the E matmul: out = lhsT.T @ rhs must be blkdiag(E, E) @ rhs
    c_e = np.zeros((128, 128), dtype=np.float32)
    c_e[0:64, 0:64] = e.T.astype(np.float32)
    c_e[64:128, 64:128] = e.T.astype(np.float32)

    c_i = np.eye(128, dtype=np.float32)

    # Banded reduction weights: slicing cols [63-2s : 127-2s] of this gives a
    # [128, 64] lhsT whose only nonzero columns are 2s (partitions 0:64) and
    # 2s+1 (partitions 64:128) — so supertile s's partition-sums land in PSUM
    # rows 2s, 2s+1 while start=False accumulation leaves other rows untouched.
    # bf16: ones are exact, and bf16 matmuls stream 4x faster than fp32.
    import ml_dtypes

    c_ones = np.zeros((128, 128), dtype=ml_dtypes.bfloat16)
    for p in range(128):
        c_ones[p, 63 + p // 64] = 1.0

    # Boundary fold: sum((perm + a 1^T) (.) E p) = sum(perm (.) E p) + g^T rowsums(p)
    # when E^T a = g, so the Neumann boundary terms ride the same product/reduce.
    a = np.linalg.lstsq(e.T, g, rcond=None)[0]
    assert np.abs(e.T @ a - g).max() < 1e-9
    c_a = np.zeros((128, 2), dtype=np.float32)
    c_a[:, 0] = np.tile(a, 2).astype(np.float32)
    c_a[:, 1] = -c_a[:, 0]

    return {"cE": c_e, "cI": c_i, "cOnes": c_ones, "cA": c_a}


def _build_nc():
    nc = bacc.Bacc("TRN2", target_bir_lowering=False, debug=False)

    x0 = nc.dram_tensor("x0", [NS, 2, 128, 512], F32, kind="ExternalInput")
    mo = nc.dram_tensor("mo", [NCHUNK, 128, CHUNK_F], F32, kind="ExternalInput")
    tg = nc.dram_tensor("tg", [NCHUNK, 128, CHUNK_F], F32, kind="ExternalInput")
    c_e = nc.dram_tensor("cE", [128, 128], F32, kind="ExternalInput")
    c_i = nc.dram_tensor("cI", [128, 128], F32, kind="ExternalInput")
    c_ones = nc.dram_tensor(
        "cOnes", [128, 128], mybir.dt.bfloat16, kind="ExternalInput"
    )
    c_a = nc.dram_tensor("cA", [128, 2], F32, kind="ExternalInput")

    s1_out = nc.dram_tensor("s1", [64, 8], F32, kind="ExternalOutput")
    s2_out = nc.dram_tensor("s2", [64, 8], F32, kind="ExternalOutput")
    dstat_out = nc.dram_tensor("dstat", [128, NCHUNK], F32, kind="ExternalOutput")

    with tile.TileContext(nc) as tc:
        with (
            tc.tile_pool(name="consts", bufs=1) as cpool,
            tc.tile_pool(name="inp", bufs=4) as ipool,
            tc.tile_pool(name="work", bufs=2) as wpool,
            tc.tile_pool(name="dchunk", bufs=4) as dpool,
            tc.tile_pool(name="stats", bufs=1) as stpool,
            tc.tile_pool(name="pwork", bufs=2, space="PSUM") as pwpool,
            tc.tile_pool(name="ptrans", bufs=1, space="PSUM") as ptpool,
            tc.tile_pool(name="paccum", bufs=1, space="PSUM") as papool,
        ):
            ce = cpool.tile([128, 128], F32, tag="ce")
            ci = cpool.tile([128, 128], F32, tag="ci")
            cones = cpool.tile([128, 128], mybir.dt.bfloat16, tag="cones")
            ca = cpool.tile([128, 2], F32, tag="ca")
            nc.sync.dma_start(ce[:], c_e[:])
            nc.sync.dma_start(ci[:], c_i[:])
            nc.sync.dma_start(cones[:], c_ones[:])
            nc.sync.dma_start(ca[:], c_a[:])

            sall = papool.tile([64, 512], F32, tag="sall")
            st = papool.tile([64, 512], F32, tag="st")
            dstat = stpool.tile([128, NCHUNK], F32, tag="dstat")

            for s in range(NS):
                p_t = ipool.tile([128, 512], F32, tag="p")
                perm_t = ipool.tile([128, 512], F32, tag="perm")
                # supertile layout: partition 64*r + h, free 64*j + w holds
                # batch 16*s + 8*r + j (channel 0 -> p_t, channel 1 -> perm_t)
                nc.sync.dma_start(p_t[:], x0[s, 0])
                nc.sync.dma_start(perm_t[:], x0[s, 1])

                ptp = ptpool.tile([128, 512], F32, tag="ptp")
                permtp = ptpool.tile([128, 512], F32, tag="permtp")
                for k in range(4):
                    nc.tensor.transpose(
                        ptp[:, 128 * k : 128 * (k + 1)],
                        p_t[:, 128 * k : 128 * (k + 1)],
                        ci[:],
                    )
                    nc.tensor.transpose(
                        permtp[:, 128 * k : 128 * (k + 1)],
                        perm_t[:, 128 * k : 128 * (k + 1)],
                        ci[:],
                    )
                pt_s = wpool.tile([128, 512], F32, tag="pt")
                permt_s = wpool.tile([128, 512], F32, tag="permt")
                nc.scalar.copy(pt_s[:], ptp[:])
                nc.scalar.copy(permt_s[:], permtp[:])

                ep = pwpool.tile([128, 512], F32, tag="ep")
                ept = pwpool.tile([128, 512], F32, tag="ept")
                nc.tensor.matmul(ep[:], ce[:], p_t[:], start=True, stop=True)
                nc.tensor.matmul(ept[:], ce[:], pt_s[:], start=True, stop=True)

                u1 = wpool.tile([128, 512], mybir.dt.bfloat16, tag="u1")
                u2 = wpool.tile([128, 512], mybir.dt.bfloat16, tag="u2")
                nc.vector.scalar_tensor_tensor(
                    u1[:], perm_t[:], ca[:, 0:1], ep[:],
                    op0=mybir.AluOpType.add, op1=mybir.AluOpType.mult,
                )
                nc.vector.scalar_tensor_tensor(
                    u2[:], permt_s[:], ca[:, 1:2], ept[:],
                    op0=mybir.AluOpType.add, op1=mybir.AluOpType.mult,
                )

                # per-(map,col) partition sums accumulated into persistent PSUM
                # rows 2s, 2s+1 via the banded lhsT slice
                lo, hi = 63 - 2 * s, 127 - 2 * s
                first, last = s == 0, s == NS - 1
                nc.tensor.matmul(
                    sall[:], cones[:, lo:hi], u1[:],
                    start=first, stop=last, skip_group_check=True,
                )
                nc.tensor.matmul(
                    st[:], cones[:, lo:hi], u2[:],
                    start=first, stop=last, skip_group_check=True,
                )

                # data loss: one [128, 2048] chunk every other supertile;
                # subtract alternates DVE/GpSimd to balance engine load
                if s % 2 == 0:
                    k = s // 2
                    mt = dpool.tile([128, CHUNK_F], F32, tag="mt")
                    tt = dpool.tile([128, CHUNK_F], F32, tag="tt")
                    nc.sync.dma_start(mt[:], mo[k])
                    nc.sync.dma_start(tt[:], tg[k])
                    eng = nc.vector if k % 2 == 0 else nc.gpsimd
                    eng.tensor_sub(mt[:], mt[:], tt[:])
                    nc.scalar.activation(
                        mt[:],
                        mt[:],
                        mybir.ActivationFunctionType.Square,
                        accum_out=dstat[:, k : k + 1],
                    )

            s1_t = stpool.tile([64, 8], F32, tag="s1t")
            s2_t = stpool.tile([64, 8], F32, tag="s2t")
            nc.vector.reduce_sum(
                s1_t[:],
                sall[:].rearrange("p (j w) -> p j w", j=8),
                axis=mybir.AxisListType.X,
            )
            nc.vector.reduce_sum(
                s2_t[:],
                st[:].rearrange("p (j w) -> p j w", j=8),
                axis=mybir.AxisListType.X,
            )
            nc.sync.dma_start(s1_out[:], s1_t[:])
            nc.sync.dma_start(s2_out[:], s2_t[:])
            nc.sync.dma_start(dstat_out[:], dstat[:])

    nc.compile()
    return nc


_NC = None
_CONSTS = None
LAST_RESULTS = None


def kernel(model_out, target, x0_hat, var, _trace=False, _trace_kwargs=None):
    global _NC, _CONSTS, LAST_RESULTS
    if _NC is None:
        _CONSTS = _build_consts()
        _NC = _build_nc()

    model_out = np.ascontiguousarray(model_out, dtype=np.float32)
    target = np.ascontiguousarray(target, dtype=np.float32)
    x0_hat = np.ascontiguousarray(x0_hat, dtype=np.float32)
    var = np.asarray(var, dtype=np.float32)

    in_maps = []
    for c in range(N_CORES):
        lo, hi = c * BPC, (c + 1) * BPC
        # pre-arrange x0 into supertile layout so the device DMA reads are
        # contiguous: out[s, ch, 64r+h, 64j+w] = x0[lo + 16s+8r+j, ch, h, w]
        x0_arr = np.ascontiguousarray(
            x0_hat[lo:hi]
            .reshape(NS, 2, 8, 2, H, W)
            .transpose(0, 3, 1, 4, 2, 5)
        ).reshape(NS, 2, 128, 512)
        in_maps.append(
            {
                "x0": x0_arr,
                "mo": model_out[lo:hi].reshape(NCHUNK, 128, CHUNK_F),
                "tg": target[lo:hi].reshape(NCHUNK, 128, CHUNK_F),
                **_CONSTS,
            }
        )

    kwargs = {}
    if _trace:
        kwargs["trace"] = True
        if _trace_kwargs:
            kwargs.update(_trace_kwargs)
    res = run_bass_kernel_spmd(_NC, in_maps, list(range(N_CORES)), **kwargs)
    LAST_RESULTS = res

    data_sum = 0.0
    nll_sum = 0.0
    for c in range(N_CORES):
        out = res.results[c]
        s1 = out["s1"].astype(np.float64)       # [64, 8]
        s2 = out["s2"].astype(np.float64)       # [64, 8]
        dstat = out["dstat"].astype(np.float64)  # [128, 16]

        # s1[2s+r, j] -> batch 16s + 8r + j
        r1 = s1.reshape(NS, 2, 8).reshape(BPC)
        # s2[2s+x, 2k+y] -> batch 16s + 8y + 2k + x
        r2 = s2.reshape(NS, 2, 4, 2).transpose(0, 3, 2, 1).reshape(BPC)
        r = (r1 + r2) / (H * W * 3.0)

        v = var[c * BPC : (c + 1) * BPC].astype(np.float64)
        nll = np.minimum(0.5 * r * r / v, CLAMP_NEG_MIN)
        nll_sum += nll.sum()
        data_sum += dstat.sum()

    loss = data_sum / (B * C * H * W) + nll_sum / B
    return np.float32(loss)


# revision 10
# speedup vs baseline: 7.0843x; 1.6559x over previous
import sys

if "/opt/trn_rl_repo" not in sys.path:
    sys.path.insert(0, "/opt/trn_rl_repo")

import numpy as np

from concourse import bacc, mybir, tile
from concourse.bass_utils import run_bass_kernel_spmd

N_CORES = 8
B, C, H, W = 4096, 2, 64, 64
BPC = B // N_CORES          # 512 batches per core
NS = BPC // 16              # 32 supertiles of 16 maps each
NCHUNK = 16                 # data-loss chunks of [128, 2048] per tensor
CHUNK_F = 2048
GRID_D = 1.0 / (H - 1)
CLAMP_NEG_MIN = 27.6310211159  # -CLAMP_MIN

F32 = mybir.dt.float32
BF16 = mybir.dt.bfloat16


def _d1_matrix(n, d):
    m = np.zeros((n, n), dtype=np.float64)
    for i in range(1, n - 1):
        m[i, i - 1], m[i, i + 1] = -1.0, 1.0
    m[0, 0], m[0, 1], m[0, 2] = -3.0, 4.0, -1.0
    m[-1, -1], m[-1, -2], m[-1, -3] = 3.0, -4.0, 1.0
    return m / (2.0 * d)


def _d2_matrix(n, d):
    m = np.zeros((n, n), dtype=np.float64)
    for i in range(1, n - 1):
        m[i, i - 1], m[i, i], m[i, i + 1] = 1.0, -2.0, 1.0
    m[0, 0:4] = [2.0, -5.0, 4.0, -1.0]
    m[-1, -1], m[-1, -2], m[-1, -3], m[-1, -4] = 2.0, -5.0, 4.0, -1.0
    return m / (d * d)


def _build_consts():
    d1 = _d1_matrix(H, GRID_D)
    d2 = _d2_matrix(H, GRID_D)
    e = -(d2 + d1.T @ d1)            # sum(perm*(E@p)) == -sum(perm*d2p) - sum(d1perm*d1p)
    g = d1[H - 1, :] - d1[0, :]      # Neumann-boundary row functional

    import ml_dtypes

    # lhsT for the E matmul: out = lhsT.T @ rhs must be blkdiag(E, E) @ rhs
    c_e = np.zeros((128, 128), dtype=ml_dtypes.bfloat16)
    c_e[0:64, 0:64] = e.T.astype(ml_dtypes.bfloat16)
    c_e[64:128, 64:128] = e.T.astype(ml_dtypes.bfloat16)

    c_i = np.eye(128, dtype=ml_dtypes.bfloat16)

    # Banded reduction weights: slicing cols [63-2s : 127-2s] of this gives a
    # [128, 64] lhsT whose only nonzero columns are 2s (partitions 0:64) and
    # 2s+1 (partitions 64:128) — so supertile s's partition-sums land in PSUM
    # rows 2s, 2s+1 while start=False accumulation leaves other rows untouched.
    # bf16: ones are exact, and bf16 matmuls stream 4x faster than fp32.
    c_ones = np.zeros((128, 128), dtype=ml_dtypes.bfloat16)
    for p in range(128):
        c_ones[p, 63 + p // 64] = 1.0

    # Boundary fold: sum((perm + a 1^T) (.) E p) = sum(perm (.) E p) + g^T rowsums(p)
    # when E^T a = g, so the Neumann boundary terms ride the same product/reduce.
    a = np.linalg.lstsq(e.T, g, rcond=None)[0]
    assert np.abs(e.T @ a - g).max() < 1e-9
    c_a = np.zeros((128, 2), dtype=np.float32)
    c_a[:, 0] = np.tile(a, 2).astype(np.float32)
    c_a[:, 1] = -c_a[:, 0]

    return {"cE": c_e, "cI": c_i, "cOnes": c_ones, "cA": c_a}


def _build_nc():
    nc = bacc.Bacc("TRN2", target_bir_lowering=False, debug=False)

    x0 = nc.dram_tensor("x0", [NS, 2, 128, 512], BF16, kind="ExternalInput")
    mo = nc.dram_tensor("mo", [NCHUNK, 128, CHUNK_F], BF16, kind="ExternalInput")
    tg = nc.dram_tensor("tg", [NCHUNK, 128, CHUNK_F], BF16, kind="ExternalInput")
    c_e = nc.dram_tensor("cE", [128, 128], BF16, kind="ExternalInput")
    c_i = nc.dram_tensor("cI", [128, 128], BF16, kind="ExternalInput")
    c_ones = nc.dram_tensor(
        "cOnes", [128, 128], mybir.dt.bfloat16, kind="ExternalInput"
    )
    c_a = nc.dram_tensor("cA", [128, 2], F32, kind="ExternalInput")

    s1_out = nc.dram_tensor("s1", [64, 8], F32, kind="ExternalOutput")
    s2_out = nc.dram_tensor("s2", [64, 8], F32, kind="ExternalOutput")
    dstat_out = nc.dram_tensor("dstat", [128, NCHUNK], F32, kind="ExternalOutput")

    with tile.TileContext(nc) as tc:
        with (
            tc.tile_pool(name="consts", bufs=1) as cpool,
            tc.tile_pool(name="inp", bufs=4) as ipool,
            tc.tile_pool(name="work", bufs=2) as wpool,
            tc.tile_pool(name="dchunk", bufs=4) as dpool,
            tc.tile_pool(name="stats", bufs=1) as stpool,
            tc.tile_pool(name="pwork", bufs=2, space="PSUM") as pwpool,
            tc.tile_pool(name="ptrans", bufs=1, space="PSUM") as ptpool,
            tc.tile_pool(name="paccum", bufs=1, space="PSUM") as papool,
        ):
            ce = cpool.tile([128, 128], BF16, tag="ce")
            ci = cpool.tile([128, 128], BF16, tag="ci")
            cones = cpool.tile([128, 128], BF16, tag="cones")
            ca = cpool.tile([128, 2], F32, tag="ca")
            nc.sync.dma_start(ce[:], c_e[:])
            nc.sync.dma_start(ci[:], c_i[:])
            nc.sync.dma_start(cones[:], c_ones[:])
            nc.sync.dma_start(ca[:], c_a[:])

            sall = papool.tile([64, 512], F32, tag="sall")
            st = papool.tile([64, 512], F32, tag="st")
            dstat = stpool.tile([128, NCHUNK], F32, tag="dstat")

            for s in range(NS):
                p_t = ipool.tile([128, 512], BF16, tag="p")
                perm_t = ipool.tile([128, 512], BF16, tag="perm")
                # supertile layout: partition 64*r + h, free 64*j + w holds
                # batch 16*s + 8*r + j (channel 0 -> p_t, channel 1 -> perm_t)
                nc.sync.dma_start(p_t[:], x0[s, 0])
                nc.sync.dma_start(perm_t[:], x0[s, 1])

                ptp = ptpool.tile([128, 512], BF16, tag="ptp")
                permtp = ptpool.tile([128, 512], BF16, tag="permtp")
                for k in range(4):
                    nc.tensor.transpose(
                        ptp[:, 128 * k : 128 * (k + 1)],
                        p_t[:, 128 * k : 128 * (k + 1)],
                        ci[:],
                    )
                    nc.tensor.transpose(
                        permtp[:, 128 * k : 128 * (k + 1)],
                        perm_t[:, 128 * k : 128 * (k + 1)],
                        ci[:],
                    )
                pt_s = wpool.tile([128, 512], BF16, tag="pt")
                permt_s = wpool.tile([128, 512], BF16, tag="permt")
                nc.scalar.copy(pt_s[:], ptp[:])
                nc.scalar.copy(permt_s[:], permtp[:])

                ep = pwpool.tile([128, 512], F32, tag="ep")
                ept = pwpool.tile([128, 512], F32, tag="ept")
                nc.tensor.matmul(ep[:], ce[:], p_t[:], start=True, stop=True)
                nc.tensor.matmul(ept[:], ce[:], pt_s[:], start=True, stop=True)

                u1 = wpool.tile([128, 512], BF16, tag="u1")
                u2 = wpool.tile([128, 512], BF16, tag="u2")
                nc.vector.scalar_tensor_tensor(
                    u1[:], perm_t[:], ca[:, 0:1], ep[:],
                    op0=mybir.AluOpType.add, op1=mybir.AluOpType.mult,
                )
                nc.vector.scalar_tensor_tensor(
                    u2[:], permt_s[:], ca[:, 1:2], ept[:],
                    op0=mybir.AluOpType.add, op1=mybir.AluOpType.mult,
                )

                # per-(map,col) partition sums accumulated into persistent PSUM
                # rows 2s, 2s+1 via the banded lhsT slice
                lo, hi = 63 - 2 * s, 127 - 2 * s
                first, last = s == 0, s == NS - 1
                nc.tensor.matmul(
                    sall[:], cones[:, lo:hi], u1[:],
                    start=first, stop=last, skip_group_check=True,
                )
                nc.tensor.matmul(
                    st[:], cones[:, lo:hi], u2[:],
                    start=first, stop=last, skip_group_check=True,
                )

                # data loss: one [128, 2048] chunk every other supertile;
                # subtract alternates DVE/GpSimd to balance engine load
                if s % 2 == 0:
                    k = s // 2
                    mt = dpool.tile([128, CHUNK_F], BF16, tag="mt")
                    tt = dpool.tile([128, CHUNK_F], BF16, tag="tt")
                    nc.sync.dma_start(mt[:], mo[k])
                    nc.sync.dma_start(tt[:], tg[k])
                    eng = nc.vector if k % 2 == 0 else nc.gpsimd
                    eng.tensor_sub(mt[:], mt[:], tt[:])
                    nc.scalar.activation(
                        mt[:],
                        mt[:],
                        mybir.ActivationFunctionType.Square,
                        accum_out=dstat[:, k : k + 1],
                    )

            s1_t = stpool.tile([64, 8], F32, tag="s1t")
            s2_t = stpool.tile([64, 8], F32, tag="s2t")
            nc.vector.reduce_sum(
                s1_t[:],
                sall[:].rearrange("p (j w) -> p j w", j=8),
                axis=mybir.AxisListType.X,
            )
            nc.vector.reduce_sum(
                s2_t[:],
                st[:].rearrange("p (j w) -> p j w", j=8),
                axis=mybir.AxisListType.X,
            )
            nc.sync.dma_start(s1_out[:], s1_t[:])
            nc.sync.dma_start(s2_out[:], s2_t[:])
            nc.sync.dma_start(dstat_out[:], dstat[:])

    nc.compile()
    return nc


_NC = None
_CONSTS = None
LAST_RESULTS = None


def kernel(model_out, target, x0_hat, var, _trace=False, _trace_kwargs=None):
    global _NC, _CONSTS, LAST_RESULTS
    if _NC is None:
        _CONSTS = _build_consts()
        _NC = _build_nc()

    import ml_dtypes

    bf = ml_dtypes.bfloat16
    model_out = np.asarray(model_out).astype(bf)
    target = np.asarray(target).astype(bf)
    x0_hat = np.asarray(x0_hat, dtype=np.float32)
    var = np.asarray(var, dtype=np.float32)

    in_maps = []
    for c in range(N_CORES):
        lo, hi = c * BPC, (c + 1) * BPC
        # pre-arrange x0 into supertile layout so the device DMA reads are
        # contiguous: out[s, ch, 64r+h, 64j+w] = x0[lo + 16s+8r+j, ch, h, w]
        x0_arr = (
            x0_hat[lo:hi]
            .reshape(NS, 2, 8, 2, H, W)
            .transpose(0, 3, 1, 4, 2, 5)
            .astype(bf)
            .reshape(NS, 2, 128, 512)
        )
        in_maps.append(
            {
                "x0": x0_arr,
                "mo": model_out[lo:hi].reshape(NCHUNK, 128, CHUNK_F),
                "tg": target[lo:hi].reshape(NCHUNK, 128, CHUNK_F),
                **_CONSTS,
            }
        )

    kwargs = {}
    if _trace:
        kwargs["trace"] = True
        if _trace_kwargs:
            kwargs.update(_trace_kwargs)
    res = run_bass_kernel_spmd(_NC, in_maps, list(range(N_CORES)), **kwargs)
    LAST_RESULTS = res

    data_sum = 0.0
    nll_sum = 0.0
    for c in range(N_CORES):
        out = res.results[c]
        s1 = out["s1"].astype(np.float64)       # [64, 8]
        s2 = out["s2"].astype(np.float64)       # [64, 8]
        dstat = out["dstat"].astype(np.float64)  # [128, 16]

        # s1[2s+r, j] -> batch 16s + 8r + j
        r1 = s1.reshape(NS, 2, 8).reshape(BPC)
        # s2[2s+x, 2k+y] -> batch 16s + 8y + 2k + x
        r2 = s2.reshape(NS, 2, 4, 2).transpose(0, 3, 2, 1).reshape(BPC)
        r = (r1 + r2) / (H * W * 3.0)

        v = var[c * BPC : (c + 1) * BPC].astype(np.float64)
        nll = np.minimum(0.5 * r * r / v, CLAMP_NEG_MIN)
        nll_sum += nll.sum()
        data_sum += dstat.sum()

    loss = data_sum / (B * C * H * W) + nll_sum / B
    return np.float32(loss)
